# revision 25
# baseline (speedup 1.0000x reference)
"""Trainium2 Bass kernel for prefix-LM CausalSelfAttention.

Problem: B=2, T=2048, C=2048, H=16 heads (hd=128), prefix-LM mask
(bidirectional over first half, causal after), RoPE on q/k.

Sharding over 8 cores: data-parallel on batch (2) x tensor-parallel on
heads (4 heads per core). Each core computes a partial output projection
(its heads' contribution); partials are summed on host.

Weight/activation matmul operands are bf16 (host-cast): bf16 stationary
operands get the compiler's fast-weight-load, which halves the
per-matmul weight-load overhead vs f32r (LDWEIGHTS 173ns -> 83ns).
Softmax probabilities stay float32r: ACT-written bf16 tiles stream
~40% slower as the MOVING matmul operand on this silicon, while f32r
moving operands with free dim >= 256 run at the full 1 row/cycle rate.
PSUM accumulation is always f32.

Per-core dataflow:
  A. qT/kT = W_{q,k}^T @ x^T   [hd*4, T] transposed layout (head-major).
     x^T is loaded once as 16 full-T k-tiles and reused by stage C
     (fewer, bigger DMAs; each dma_start costs ~0.6us of Sync-sequencer
     issue time).
  B. RoPE via pair-swap permutation matmul + DVE combine, interleaved
     with (C) so the PE stays fed while DVE does the rope math
  C. v = x @ Wv   [T, hd*4] natural layout (x tiles as stationary)
  D. attention, query-chunk outer / head inner; per 512-wide chunk I:
       S'[J] = k_rope[:,J]^T-tile x q_rope[:,I]    (scores transposed)
       P'[J] = exp(S' / sqrt(hd))                  (ACT, PSUM->SBUF f32r)
       masked diagonal tiles multiplied by static patterns (DVE) and
       trimmed to their allowed query sub-range (columns >= 128*dmi)
       y_psum += v[J,h]^T-as-lhsT x P'[J]
       d_psum += ones^T x P'[J]                    (denominator)
     normalize y^T by broadcast fast-reciprocal of d
     (reciprocal_approx_fast: ~5x faster than DVE reciprocal, 18-bit)
  E. partial out = yT^T-as-lhsT @ Wp; E work is interleaved into the
     NEXT query chunk's attention stream so it fills the PE stalls left
     by the S->exp->PV latency chain; out rows are staged in SBUF and
     DMA'd once per 128-token row block.

Fully-masked key tiles are skipped (structural sparsity: 44/64 tiles/head).
"""
import math

import numpy as np

N_HEAD = 16
B = 2
T = 2048
C = 2048
HD = 128
HPC = 4          # heads per core
CL = HPC * HD    # local C = 512
TC = 512         # chunk width (matmul moving free dim / psum bank)
NT = T // TC     # 4 chunks
KT = C // 128    # 16 contraction tiles over C
TT = T // 128    # 16 T tiles
SCALE = 1.0 / math.sqrt(HD)

# Per query-chunk I: list of (J, mask_idx) key tiles to compute.
_JLISTS = {
    0: [(j, None) for j in range(8)],
    1: [(j, None) for j in range(8)],
    2: [(j, None) for j in range(8)] + [(8 + d, d) for d in range(4)],
    3: [(j, None) for j in range(12)] + [(12 + d, d) for d in range(4)],
}

_CACHE = {}


def _build_nc():
    import concourse.tile as tile
    import concourse.mybir as mybir
    from concourse import bacc

    f32 = mybir.dt.float32
    bf16 = mybir.dt.bfloat16

    nc = bacc.Bacc(None, target_bir_lowering=False)

    xT = nc.dram_tensor("xT", [C, T], bf16, kind="ExternalInput")
    wqk = nc.dram_tensor("wqk", [C, 2 * CL], bf16, kind="ExternalInput")
    wv = nc.dram_tensor("wv", [C, CL], bf16, kind="ExternalInput")
    wp = nc.dram_tensor("wp", [CL, C], bf16, kind="ExternalInput")
    cosP = nc.dram_tensor("cosP", [HD, T], bf16, kind="ExternalInput")
    sinP = nc.dram_tensor("sinP", [HD, T], bf16, kind="ExternalInput")
    f32r = mybir.dt.float32r
    rt = nc.dram_tensor("rt", [HD, HD], bf16, kind="ExternalInput")
    masks = nc.dram_tensor("masks", [4, 128, TC], f32r, kind="ExternalInput")
    ones = nc.dram_tensor("ones", [128, 1], f32r, kind="ExternalInput")
    out = nc.dram_tensor("out", [T, C], f32, kind="ExternalOutput")

    xT3 = xT.rearrange("(kt p) t -> p kt t", p=128)
    wqk3 = wqk.rearrange("(kt p) m -> p kt m", p=128)
    wv3 = wv.rearrange("(kt p) m -> p kt m", p=128)
    wp3 = wp.rearrange("(kt p) m -> p kt m", p=128)
    masks3 = masks.rearrange("d p n -> p d n")

    Exp = mybir.ActivationFunctionType.Exp

    with tile.TileContext(nc) as tc:
        mpool = tc.alloc_tile_pool(name="misc", bufs=1)
        qk_pool = tc.alloc_tile_pool(name="qkrope", bufs=1)
        tpool = tc.alloc_tile_pool(name="trig", bufs=1, side="right")
        xpool = tc.alloc_tile_pool(name="xt_sb", bufs=1, side="right")

        rt_sb = mpool.tile([HD, HD], bf16)
        ones_sb = mpool.tile([128, 1], f32r)
        mask_sb = mpool.tile([128, 4, TC], f32r)
        cos_sb = tpool.tile([HD, T], bf16)
        sin_sb = tpool.tile([HD, T], bf16)
        warm_sb = mpool.tile([1, 1], f32)

        qkT = [qk_pool.tile([128, T], bf16, tag=f"qk{m}", name=f"qk{m}") for m in range(8)]

        # ---- stage A: qT/kT = W_{q,k}^T @ x^T, head-major tiles ----
        # x^T k-tiles are full-T and stay resident through stage C.
        wpool = tc.alloc_tile_pool(name="wqk_sb", bufs=1)
        ps1 = tc.alloc_tile_pool(name="ps_qk", bufs=8, space="PSUM")
        w_t = []
        x_t = []
        for k in range(KT):
            wt = wpool.tile([128, 2 * CL], bf16, tag=f"w{k}", name=f"w{k}")
            # m=0 slice first: the opening PSUM group only needs cols 0:128
            # of every w k-tile, so its critical DMA set shrinks 6MB->2.5MB
            nc.sync.dma_start(out=wt[:, 0:128], in_=wqk3[:, k, 0:128])
            w_t.append(wt)
            xt = xpool.tile([128, T], bf16, tag=f"x{k}", name=f"x{k}")
            nc.sync.dma_start(out=xt[:, 0:TC], in_=xT3[:, k, 0:TC])
            x_t.append(xt)
        for k in range(KT):
            nc.sync.dma_start(out=w_t[k][:, 128:], in_=wqk3[:, k, 128:])
        for k in range(KT):
            nc.sync.dma_start(out=x_t[k][:, TC:], in_=xT3[:, k, TC:])
        nc.sync.dma_start(out=rt_sb, in_=rt[:, :])
        nc.sync.dma_start(out=cos_sb, in_=cosP[:, :])
        nc.sync.dma_start(out=sin_sb, in_=sinP[:, :])
        # warm the ACT exp table set during stage A (one-time ~2.7us load)
        nc.scalar.activation(out=warm_sb, in_=rt_sb[0:1, 0:1], func=Exp)
        for n in range(NT):
            nsl = slice(n * TC, (n + 1) * TC)
            for m in range(8):
                ps = ps1.tile([128, TC], f32, tag="ps_qk", name="ps_qk")
                for k in range(KT):
                    nc.tensor.matmul(
                        ps, w_t[k][:, m * 128:(m + 1) * 128], x_t[k][:, nsl],
                        start=(k == 0), stop=(k == KT - 1),
                    )
                nc.vector.tensor_copy(out=qkT[m][:, nsl], in_=ps)
        wpool.release()
        ps1.release()

        # ---- stage B+C interleaved: RoPE (PE tiny, DVE heavy) and
        # v = x @ Wv (PE heavy). Emitting v matmuls after each head's rope
        # keeps the PE busy while DVE works through the rope muls.
        v_pool = tc.alloc_tile_pool(name="v_sb", bufs=1)
        wvpool = tc.alloc_tile_pool(name="wv_sb", bufs=1)
        v_t = [v_pool.tile([128, CL], f32r, tag=f"v{mt}", name=f"v{mt}")
               for mt in range(TT)]
        wv_t = []
        for k in range(KT):
            wt = wvpool.tile([128, CL], bf16, tag=f"wv{k}", name=f"wv{k}")
            nc.sync.dma_start(out=wt, in_=wv3[:, k])
            wv_t.append(wt)

        rope = [None] * 8
        rtmp = tc.alloc_tile_pool(name="rope_tmp", bufs=4)
        psr = tc.alloc_tile_pool(name="ps_rot", bufs=4, space="PSUM")
        ps2 = tc.alloc_tile_pool(name="ps_v", bufs=4, space="PSUM")
        nc.sync.dma_start(out=ones_sb, in_=ones[:, :])
        nc.sync.dma_start(out=mask_sb, in_=masks3)

        def emit_v_pair(pair):
            for half in range(2):
                mt = 2 * pair + half
                tsl = slice(mt * 128, (mt + 1) * 128)
                ps = ps2.tile([128, CL], f32, tag="ps_v", name="ps_v")
                for k in range(KT):
                    nc.tensor.matmul(
                        ps, x_t[k][:, tsl], wv_t[k],
                        start=(k == 0), stop=(k == KT - 1),
                    )
                nc.scalar.copy(out=v_t[mt], in_=ps)

        for idx, m in enumerate((0, 4, 1, 5, 2, 6, 3, 7)):
            tmp = []
            for n in range(NT):
                sl = slice(n * TC, (n + 1) * TC)
                ps = psr.tile([128, TC], f32, tag="ps_rot", name="ps_rot")
                nc.tensor.matmul(ps, rt_sb, qkT[m][:, sl], start=True, stop=True)
                t1 = rtmp.tile([128, TC], bf16, tag="t1", name="t1")
                t2 = rtmp.tile([128, TC], bf16, tag="t2", name="t2")
                nc.vector.tensor_mul(t1, ps, sin_sb[:, sl])
                nc.vector.tensor_mul(t2, qkT[m][:, sl], cos_sb[:, sl])
                tmp.append((t1, t2))
            ro = qk_pool.tile([128, T], bf16, tag=f"qk{m}", name=f"rope{m}")
            for n in range(NT):
                sl = slice(n * TC, (n + 1) * TC)
                nc.vector.tensor_add(ro[:, sl], tmp[n][0], tmp[n][1])
            rope[m] = ro
            emit_v_pair(idx)
        ps2.release()
        psr.release()
        rtmp.release()
        wvpool.release()
        xpool.release()
        tpool.release()

        # ---- stage D attention (query-chunk outer) + stage E interleaved ----
        y_pool = tc.alloc_tile_pool(name="yT_sb", bufs=1)
        yT = [y_pool.tile([128, T], bf16, tag=f"yT{h}", name=f"yT{h}")
              for h in range(HPC)]
        wppool = tc.alloc_tile_pool(name="wp_sb", bufs=1)
        wp_t = []
        for hk in range(HPC):
            wt = wppool.tile([128, C], bf16, tag=f"wp{hk}", name=f"wp{hk}")
            nc.sync.dma_start(out=wt, in_=wp3[:, hk])
            wp_t.append(wt)

        pp_pool = tc.alloc_tile_pool(name="pp", bufs=6)
        sm_pool = tc.alloc_tile_pool(name="small", bufs=2)
        ps_s = tc.alloc_tile_pool(name="ps_s", bufs=4, space="PSUM")
        ps_y = tc.alloc_tile_pool(name="ps_y", bufs=2, space="PSUM")
        ps_o = tc.alloc_tile_pool(name="ps_o", bufs=1, space="PSUM")
        ps_d = tc.alloc_tile_pool(name="ps_d", bufs=1, space="PSUM")  # two halves
        opool = tc.alloc_tile_pool(name="ostage", bufs=2)

        ready_E = []     # mt values whose yT inputs are complete
        e_state = [None, 0]  # open [ot_tile, next_n] for current mt

        e_pools = [ps_o]
        e_ctr = [0]

        def emit_e_subgroup():
            # one (mt, n) block: 4 accumulating matmuls + copy to the
            # staged out row; DMA the full row after its 4th block.
            if e_state[0] is None:
                if not ready_E:
                    return
                e_state[0] = (ready_E.pop(0),
                              opool.tile([128, C], f32, tag="ot", name="ot"))
                e_state[1] = 0
            mt, ot = e_state[0]
            n = e_state[1]
            msl = slice(mt * 128, (mt + 1) * 128)
            pool = e_pools[e_ctr[0] % len(e_pools)]
            e_ctr[0] += 1
            ps = pool.tile([128, TC], f32, tag="o", name="o_ps")
            for hk in range(HPC):
                nc.tensor.matmul(
                    ps, yT[hk][:, msl], wp_t[hk][:, n * TC:(n + 1) * TC],
                    start=(hk == 0), stop=(hk == HPC - 1),
                )
            nc.vector.tensor_copy(out=ot[:, n * TC:(n + 1) * TC], in_=ps)
            if mt >= TT - 4:
                nc.sync.dma_start(out=out[msl, n * TC:(n + 1) * TC],
                                  in_=ot[:, n * TC:(n + 1) * TC])
            e_state[1] += 1
            if e_state[1] == NT:
                if mt < TT - 4:
                    nc.sync.dma_start(out=out[msl, :], in_=ot)
                e_state[0] = None

        for I in range(NT):
            isl = slice(I * TC, (I + 1) * TC)
            jl = _JLISTS[I]
            for h in range(HPC):
                q_h = rope[h]
                k_h = rope[4 + h]
                y_ps = ps_y.tile([128, TC], f32, tag="y", name="y_ps")
                d_ps = ps_d.tile([1, TC], f32, tag="d", name="d_ps")
                for jidx, (J, dmi) in enumerate(jl):
                    # diagonal tiles only attend queries >= dmi*128 within
                    # the chunk; trim the streamed range accordingly
                    off = 0 if dmi is None else dmi * 128
                    osl = slice(off, TC)
                    s_ps = ps_s.tile([128, TC], f32, tag="s", name="s_ps")
                    nc.tensor.matmul(
                        s_ps[:, osl], k_h[:, J * 128:(J + 1) * 128],
                        q_h[:, I * TC + off:(I + 1) * TC],
                        start=True, stop=True,
                    )
                    pp = pp_pool.tile([128, TC], f32r, tag="pp", name="pp")
                    nc.scalar.activation(out=pp[:, osl], in_=s_ps[:, osl],
                                         func=Exp, scale=SCALE)
                    ppv = pp[:, osl]
                    if dmi is not None:
                        ppm = pp_pool.tile([128, TC], f32r, tag="ppm",
                                           name="ppm", bufs=4)
                        nc.vector.tensor_mul(ppm[:, osl], pp[:, osl],
                                             mask_sb[:, dmi, osl])
                        ppv = ppm[:, osl]
                    first = jidx == 0
                    last = jidx == len(jl) - 1
                    nc.tensor.matmul(
                        y_ps[:, osl], v_t[J][:, h * 128:(h + 1) * 128], ppv,
                        start=first, stop=last,
                    )
                    nc.tensor.matmul(d_ps[:, osl], ones_sb, ppv,
                                     start=first, stop=last)
                    if jidx % 2 == 1 or len(ready_E) > 8:
                        emit_e_subgroup()
                recip = sm_pool.tile([1, TC], f32, tag="recip", name="recip")
                nc.vector.reciprocal_approx_fast(out=recip, in_=d_ps)
                recipB = sm_pool.tile([128, TC], f32, tag="recipB", name="recipB")
                nc.gpsimd.partition_broadcast(recipB, recip)
                nc.vector.tensor_mul(yT[h][:, isl], y_ps, recipB)
            # all heads' columns for chunk I are now complete
            ready_E.extend(range(4 * I, 4 * I + 4))
        ps_d.release()
        ps_o2 = tc.alloc_tile_pool(name="ps_o2", bufs=1, space="PSUM")
        e_pools.append(ps_o2)
        while ready_E or e_state[0] is not None:
            emit_e_subgroup()

        for p in (opool, sm_pool, pp_pool, wppool, y_pool, v_pool,
                  qk_pool, mpool, ps_o2, ps_o, ps_y, ps_s):
            p.release()
    nc.compile()
    return nc


def _host_prep(x, w_qkv, w_proj, freqs_cis):
    """Build per-core input maps (slicing + layout prep only)."""
    try:
        import ml_dtypes
        bf = ml_dtypes.bfloat16
    except ImportError:
        import jax.numpy as jnp
        bf = jnp.bfloat16
    x = np.asarray(x, dtype=np.float32)
    w_qkv = np.asarray(w_qkv, dtype=np.float32)
    w_proj = np.asarray(w_proj, dtype=np.float32)
    fc = np.asarray(freqs_cis, dtype=np.float32)

    xTb = [np.ascontiguousarray(x[b].T).astype(bf) for b in range(B)]

    cos = fc[:, :, 0].T  # [64, T]
    sin = fc[:, :, 1].T
    cosP = np.repeat(cos, 2, axis=0).astype(bf)  # [128, T]
    sinP = np.repeat(sin, 2, axis=0).astype(bf)

    rt = np.zeros((HD, HD), dtype=np.float32)
    for d in range(HD // 2):
        rt[2 * d, 2 * d + 1] = 1.0
        rt[2 * d + 1, 2 * d] = -1.0
    rt = rt.astype(bf)

    masks = np.zeros((4, 128, TC), dtype=np.float32)
    ii = np.arange(TC)[None, :]
    jj = np.arange(128)[:, None]
    for d in range(4):
        masks[d] = (ii >= jj + 128 * d).astype(np.float32)
    
    ones = np.ones((128, 1), dtype=np.float32)

    in_maps = []
    for core in range(8):
        b = core // 4
        g = core % 4
        qc = w_qkv[:, 512 * g: 512 * (g + 1)]
        kc = w_qkv[:, 2048 + 512 * g: 2048 + 512 * (g + 1)]
        vc = np.ascontiguousarray(w_qkv[:, 4096 + 512 * g: 4096 + 512 * (g + 1)]).astype(bf)
        wqk_c = np.concatenate([qc, kc], axis=1).astype(bf)
        wp_c = np.ascontiguousarray(w_proj[512 * g: 512 * (g + 1), :]).astype(bf)
        in_maps.append({
            "xT": xTb[b],
            "wqk": wqk_c,
            "wv": vc,
            "wp": wp_c,
            "cosP": cosP,
            "sinP": sinP,
            "rt": rt,
            "masks": masks,
            "ones": ones,
        })
    return in_maps


def _get_nc():
    if "nc" not in _CACHE:
        _CACHE["nc"] = _build_nc()
    return _CACHE["nc"]


def kernel(x, w_qkv, w_proj, freqs_cis, attn_mask, _trace=False):
    from concourse.bass_utils import run_bass_kernel_spmd

    in_maps = _host_prep(x, w_qkv, w_proj, freqs_cis)
    nc = _get_nc()
    res = run_bass_kernel_spmd(
        nc, in_maps, core_ids=list(range(8)), trace=_trace,
    )
    outs = [r["out"].astype(np.float64) for r in res.results]
    full = np.stack([
        outs[0] + outs[1] + outs[2] + outs[3],
        outs[4] + outs[5] + outs[6] + outs[7],
    ]).astype(np.float32)
    if _trace:
        kernel._last_results = res
    return full


# revision 26
# speedup vs baseline: 1.0357x; 1.0357x over previous
"""Trainium2 Bass kernel for prefix-LM CausalSelfAttention.

Problem: B=2, T=2048, C=2048, H=16 heads (hd=128), prefix-LM mask
(bidirectional over first half, causal after), RoPE on q/k.

Sharding over 8 cores: data-parallel on batch (2) x tensor-parallel on
heads (4 heads per core). Each core computes a partial output projection
(its heads' contribution); partials are summed on host.

Weight/activation matmul operands are bf16 (host-cast): bf16 stationary
operands get the compiler's fast-weight-load, which halves the
per-matmul weight-load overhead vs f32r (LDWEIGHTS 173ns -> 83ns).
Softmax probabilities stay float32r: ACT-written bf16 tiles stream
~40% slower as the MOVING matmul operand on this silicon, while f32r
moving operands with free dim >= 256 run at the full 1 row/cycle rate.
PSUM accumulation is always f32.

Per-core dataflow:
  A. qT/kT = W_{q,k}^T @ x^T   [hd*4, T] transposed layout (head-major).
     x^T is loaded once as 16 full-T k-tiles and reused by stage C
     (fewer, bigger DMAs; each dma_start costs ~0.6us of Sync-sequencer
     issue time).
  B. RoPE via pair-swap permutation matmul + DVE combine, interleaved
     with (C) so the PE stays fed while DVE does the rope math
  C. v = x @ Wv   [T, hd*4] natural layout (x tiles as stationary)
  D. attention, query-chunk outer / head inner; per 512-wide chunk I:
       S'[J] = k_rope[:,J]^T-tile x q_rope[:,I]    (scores transposed)
       P'[J] = exp(S' / sqrt(hd))                  (ACT, PSUM->SBUF f32r)
       masked diagonal tiles multiplied by static patterns (DVE) and
       trimmed to their allowed query sub-range (columns >= 128*dmi)
       y_psum += v[J,h]^T-as-lhsT x P'[J]
       d_psum += ones^T x P'[J]                    (denominator)
     normalize y^T by broadcast fast-reciprocal of d
     (reciprocal_approx_fast: ~5x faster than DVE reciprocal, 18-bit)
  E. partial out = yT^T-as-lhsT @ Wp; E work is interleaved into the
     NEXT query chunk's attention stream so it fills the PE stalls left
     by the S->exp->PV latency chain; out rows are staged in SBUF and
     DMA'd once per 128-token row block.

Fully-masked key tiles are skipped (structural sparsity: 44/64 tiles/head).
"""
import math

import numpy as np

N_HEAD = 16
B = 2
T = 2048
C = 2048
HD = 128
HPC = 4          # heads per core
CL = HPC * HD    # local C = 512
TC = 512         # chunk width (matmul moving free dim / psum bank)
NT = T // TC     # 4 chunks
KT = C // 128    # 16 contraction tiles over C
TT = T // 128    # 16 T tiles
SCALE = 1.0 / math.sqrt(HD)

# Per query-chunk I: list of (J, mask_idx) key tiles to compute.
_JLISTS = {
    0: [(j, None) for j in range(8)],
    1: [(j, None) for j in range(8)],
    2: [(j, None) for j in range(8)] + [(8 + d, d) for d in range(4)],
    3: [(j, None) for j in range(12)] + [(12 + d, d) for d in range(4)],
}

_CACHE = {}


def _build_nc():
    import concourse.tile as tile
    import concourse.mybir as mybir
    from concourse import bacc

    f32 = mybir.dt.float32
    bf16 = mybir.dt.bfloat16

    nc = bacc.Bacc(None, target_bir_lowering=False)

    xT = nc.dram_tensor("xT", [C, T], bf16, kind="ExternalInput")
    wqk = nc.dram_tensor("wqk", [C, 2 * CL], bf16, kind="ExternalInput")
    wv = nc.dram_tensor("wv", [C, CL], bf16, kind="ExternalInput")
    wp = nc.dram_tensor("wp", [CL, C], bf16, kind="ExternalInput")
    cosP = nc.dram_tensor("cosP", [HD, T], bf16, kind="ExternalInput")
    sinP = nc.dram_tensor("sinP", [HD, T], bf16, kind="ExternalInput")
    f32r = mybir.dt.float32r
    rt = nc.dram_tensor("rt", [HD, HD], bf16, kind="ExternalInput")
    masks = nc.dram_tensor("masks", [4, 128, TC], f32r, kind="ExternalInput")
    ones = nc.dram_tensor("ones", [128, 1], f32r, kind="ExternalInput")
    out = nc.dram_tensor("out", [T, C], f32, kind="ExternalOutput")

    xT3 = xT.rearrange("(kt p) t -> p kt t", p=128)
    wqk3 = wqk.rearrange("(kt p) m -> p kt m", p=128)
    wv3 = wv.rearrange("(kt p) m -> p kt m", p=128)
    wp3 = wp.rearrange("(kt p) m -> p kt m", p=128)
    masks3 = masks.rearrange("d p n -> p d n")

    Exp = mybir.ActivationFunctionType.Exp

    with tile.TileContext(nc) as tc:
        mpool = tc.alloc_tile_pool(name="misc", bufs=1)
        qk_pool = tc.alloc_tile_pool(name="qkrope", bufs=1)
        tpool = tc.alloc_tile_pool(name="trig", bufs=1, side="right")
        xpool = tc.alloc_tile_pool(name="xt_sb", bufs=1, side="right")

        rt_sb = mpool.tile([HD, HD], bf16)
        ones_sb = mpool.tile([128, 1], f32r)
        mask_sb = mpool.tile([128, 4, TC], f32r)
        cos_sb = tpool.tile([HD, T], bf16)
        sin_sb = tpool.tile([HD, T], bf16)
        warm_sb = mpool.tile([1, 1], f32)

        qkT = [qk_pool.tile([128, T], bf16, tag=f"qk{m}", name=f"qk{m}") for m in range(8)]

        # ---- stage A: qT/kT = W_{q,k}^T @ x^T, head-major tiles ----
        # x^T k-tiles are full-T and stay resident through stage C.
        wpool = tc.alloc_tile_pool(name="wqk_sb", bufs=1)
        ps1 = tc.alloc_tile_pool(name="ps_qk", bufs=8, space="PSUM")
        w_t = []
        x_t = []
        for k in range(KT):
            wt = wpool.tile([128, 2 * CL], bf16, tag=f"w{k}", name=f"w{k}")
            nc.sync.dma_start(out=wt, in_=wqk3[:, k])
            w_t.append(wt)
            xt = xpool.tile([128, T], bf16, tag=f"x{k}", name=f"x{k}")
            nc.sync.dma_start(out=xt[:, 0:TC], in_=xT3[:, k, 0:TC])
            x_t.append(xt)
        for k in range(KT):
            nc.sync.dma_start(out=x_t[k][:, TC:], in_=xT3[:, k, TC:])
        nc.sync.dma_start(out=rt_sb, in_=rt[:, :])
        nc.sync.dma_start(out=cos_sb, in_=cosP[:, :])
        nc.sync.dma_start(out=sin_sb, in_=sinP[:, :])
        # warm the ACT exp table set during stage A (one-time ~2.7us load)
        nc.scalar.activation(out=warm_sb, in_=rt_sb[0:1, 0:1], func=Exp)
        for n in range(NT):
            nsl = slice(n * TC, (n + 1) * TC)
            for m in range(8):
                ps = ps1.tile([128, TC], f32, tag="ps_qk", name="ps_qk")
                for k in range(KT):
                    nc.tensor.matmul(
                        ps, w_t[k][:, m * 128:(m + 1) * 128], x_t[k][:, nsl],
                        start=(k == 0), stop=(k == KT - 1),
                    )
                nc.vector.tensor_copy(out=qkT[m][:, nsl], in_=ps)
        wpool.release()
        ps1.release()

        # ---- stage B+C interleaved: RoPE (PE tiny, DVE heavy) and
        # v = x @ Wv (PE heavy). Emitting v matmuls after each head's rope
        # keeps the PE busy while DVE works through the rope muls.
        v_pool = tc.alloc_tile_pool(name="v_sb", bufs=1)
        wvpool = tc.alloc_tile_pool(name="wv_sb", bufs=1)
        v_t = [v_pool.tile([128, CL], f32r, tag=f"v{mt}", name=f"v{mt}")
               for mt in range(TT)]
        wv_t = []
        for k in range(KT):
            wt = wvpool.tile([128, CL], bf16, tag=f"wv{k}", name=f"wv{k}")
            nc.sync.dma_start(out=wt, in_=wv3[:, k])
            wv_t.append(wt)

        rope = [None] * 8
        rtmp = tc.alloc_tile_pool(name="rope_tmp", bufs=4)
        psr = tc.alloc_tile_pool(name="ps_rot", bufs=4, space="PSUM")
        ps2 = tc.alloc_tile_pool(name="ps_v", bufs=4, space="PSUM")
        nc.sync.dma_start(out=ones_sb, in_=ones[:, :])
        nc.sync.dma_start(out=mask_sb, in_=masks3)

        def emit_v_pair(pair):
            for half in range(2):
                mt = 2 * pair + half
                tsl = slice(mt * 128, (mt + 1) * 128)
                ps = ps2.tile([128, CL], f32, tag="ps_v", name="ps_v")
                for k in range(KT):
                    nc.tensor.matmul(
                        ps, x_t[k][:, tsl], wv_t[k],
                        start=(k == 0), stop=(k == KT - 1),
                    )
                nc.scalar.copy(out=v_t[mt], in_=ps)

        for idx, m in enumerate((0, 4, 1, 5, 2, 6, 3, 7)):
            tmp = []
            for n in range(NT):
                sl = slice(n * TC, (n + 1) * TC)
                ps = psr.tile([128, TC], f32, tag="ps_rot", name="ps_rot")
                nc.tensor.matmul(ps, rt_sb, qkT[m][:, sl], start=True, stop=True)
                t1 = rtmp.tile([128, TC], bf16, tag="t1", name="t1")
                t2 = rtmp.tile([128, TC], bf16, tag="t2", name="t2")
                nc.vector.tensor_mul(t1, ps, sin_sb[:, sl])
                nc.vector.tensor_mul(t2, qkT[m][:, sl], cos_sb[:, sl])
                tmp.append((t1, t2))
            ro = qk_pool.tile([128, T], bf16, tag=f"qk{m}", name=f"rope{m}")
            for n in range(NT):
                sl = slice(n * TC, (n + 1) * TC)
                nc.vector.tensor_add(ro[:, sl], tmp[n][0], tmp[n][1])
            rope[m] = ro
            emit_v_pair(idx)
        ps2.release()
        psr.release()
        rtmp.release()
        wvpool.release()
        xpool.release()
        tpool.release()

        # ---- stage D attention (query-chunk outer) + stage E interleaved ----
        y_pool = tc.alloc_tile_pool(name="yT_sb", bufs=1)
        yT = [y_pool.tile([128, T], bf16, tag=f"yT{h}", name=f"yT{h}")
              for h in range(HPC)]
        wppool = tc.alloc_tile_pool(name="wp_sb", bufs=1)
        wp_t = []
        for hk in range(HPC):
            wt = wppool.tile([128, C], bf16, tag=f"wp{hk}", name=f"wp{hk}")
            nc.sync.dma_start(out=wt, in_=wp3[:, hk])
            wp_t.append(wt)

        pp_pool = tc.alloc_tile_pool(name="pp", bufs=6)
        sm_pool = tc.alloc_tile_pool(name="small", bufs=2)
        ps_s = tc.alloc_tile_pool(name="ps_s", bufs=3, space="PSUM")
        ps_y = tc.alloc_tile_pool(name="ps_y", bufs=2, space="PSUM")
        ps_o = tc.alloc_tile_pool(name="ps_o", bufs=1, space="PSUM")
        ps_d = tc.alloc_tile_pool(name="ps_d", bufs=2, space="PSUM")  # two halves
        opool = tc.alloc_tile_pool(name="ostage", bufs=2)

        ready_E = []     # mt values whose yT inputs are complete
        e_state = [None, 0]  # open [ot_tile, next_n] for current mt

        e_pools = [ps_o]
        e_ctr = [0]

        def emit_e_subgroup():
            # one (mt, n) block: 4 accumulating matmuls + copy to the
            # staged out row; DMA the full row after its 4th block.
            if e_state[0] is None:
                if not ready_E:
                    return
                e_state[0] = (ready_E.pop(0),
                              opool.tile([128, C], f32, tag="ot", name="ot"))
                e_state[1] = 0
            mt, ot = e_state[0]
            n = e_state[1]
            msl = slice(mt * 128, (mt + 1) * 128)
            pool = e_pools[e_ctr[0] % len(e_pools)]
            e_ctr[0] += 1
            ps = pool.tile([128, TC], f32, tag="o", name="o_ps")
            for hk in range(HPC):
                nc.tensor.matmul(
                    ps, yT[hk][:, msl], wp_t[hk][:, n * TC:(n + 1) * TC],
                    start=(hk == 0), stop=(hk == HPC - 1),
                )
            nc.vector.tensor_copy(out=ot[:, n * TC:(n + 1) * TC], in_=ps)
            if mt >= TT - 4:
                nc.sync.dma_start(out=out[msl, n * TC:(n + 1) * TC],
                                  in_=ot[:, n * TC:(n + 1) * TC])
            e_state[1] += 1
            if e_state[1] == NT:
                if mt < TT - 4:
                    nc.sync.dma_start(out=out[msl, :], in_=ot)
                e_state[0] = None

        for I in range(NT):
            isl = slice(I * TC, (I + 1) * TC)
            jl = _JLISTS[I]
            for h in range(HPC):
                q_h = rope[h]
                k_h = rope[4 + h]
                y_ps = ps_y.tile([128, TC], f32, tag="y", name="y_ps")
                d_ps = ps_d.tile([1, TC], f32, tag="d", name="d_ps")
                for jidx, (J, dmi) in enumerate(jl):
                    # diagonal tiles only attend queries >= dmi*128 within
                    # the chunk; trim the streamed range accordingly
                    off = 0 if dmi is None else dmi * 128
                    osl = slice(off, TC)
                    s_ps = ps_s.tile([128, TC], f32, tag="s", name="s_ps")
                    nc.tensor.matmul(
                        s_ps[:, osl], k_h[:, J * 128:(J + 1) * 128],
                        q_h[:, I * TC + off:(I + 1) * TC],
                        start=True, stop=True,
                    )
                    pp = pp_pool.tile([128, TC], f32r, tag="pp", name="pp")
                    nc.scalar.activation(out=pp[:, osl], in_=s_ps[:, osl],
                                         func=Exp, scale=SCALE)
                    ppv = pp[:, osl]
                    if dmi is not None:
                        ppm = pp_pool.tile([128, TC], f32r, tag="ppm",
                                           name="ppm", bufs=4)
                        nc.vector.tensor_mul(ppm[:, osl], pp[:, osl],
                                             mask_sb[:, dmi, osl])
                        ppv = ppm[:, osl]
                    first = jidx == 0
                    last = jidx == len(jl) - 1
                    nc.tensor.matmul(
                        y_ps[:, osl], v_t[J][:, h * 128:(h + 1) * 128], ppv,
                        start=first, stop=last,
                    )
                    nc.tensor.matmul(d_ps[:, osl], ones_sb, ppv,
                                     start=first, stop=last)
                    if jidx % 2 == 1 or len(ready_E) > 8:
                        emit_e_subgroup()
                recip = sm_pool.tile([1, TC], f32, tag="recip", name="recip")
                nc.vector.reciprocal_approx_fast(out=recip, in_=d_ps)
                recipB = sm_pool.tile([128, TC], f32, tag="recipB", name="recipB")
                nc.gpsimd.partition_broadcast(recipB, recip)
                nc.vector.tensor_mul(yT[h][:, isl], y_ps, recipB)
            # all heads' columns for chunk I are now complete
            ready_E.extend(range(4 * I, 4 * I + 4))
        ps_d.release()
        ps_o2 = tc.alloc_tile_pool(name="ps_o2", bufs=1, space="PSUM")
        e_pools.append(ps_o2)
        while ready_E or e_state[0] is not None:
            emit_e_subgroup()

        for p in (opool, sm_pool, pp_pool, wppool, y_pool, v_pool,
                  qk_pool, mpool, ps_o2, ps_o, ps_y, ps_s):
            p.release()
    nc.compile()
    return nc


def _host_prep(x, w_qkv, w_proj, freqs_cis):
    """Build per-core input maps (slicing + layout prep only)."""
    try:
        import ml_dtypes
        bf = ml_dtypes.bfloat16
    except ImportError:
        import jax.numpy as jnp
        bf = jnp.bfloat16
    x = np.asarray(x, dtype=np.float32)
    w_qkv = np.asarray(w_qkv, dtype=np.float32)
    w_proj = np.asarray(w_proj, dtype=np.float32)
    fc = np.asarray(freqs_cis, dtype=np.float32)

    xTb = [np.ascontiguousarray(x[b].T).astype(bf) for b in range(B)]

    cos = fc[:, :, 0].T  # [64, T]
    sin = fc[:, :, 1].T
    cosP = np.repeat(cos, 2, axis=0).astype(bf)  # [128, T]
    sinP = np.repeat(sin, 2, axis=0).astype(bf)

    rt = np.zeros((HD, HD), dtype=np.float32)
    for d in range(HD // 2):
        rt[2 * d, 2 * d + 1] = 1.0
        rt[2 * d + 1, 2 * d] = -1.0
    rt = rt.astype(bf)

    masks = np.zeros((4, 128, TC), dtype=np.float32)
    ii = np.arange(TC)[None, :]
    jj = np.arange(128)[:, None]
    for d in range(4):
        masks[d] = (ii >= jj + 128 * d).astype(np.float32)
    
    ones = np.ones((128, 1), dtype=np.float32)

    in_maps = []
    for core in range(8):
        b = core // 4
        g = core % 4
        qc = w_qkv[:, 512 * g: 512 * (g + 1)]
        kc = w_qkv[:, 2048 + 512 * g: 2048 + 512 * (g + 1)]
        vc = np.ascontiguousarray(w_qkv[:, 4096 + 512 * g: 4096 + 512 * (g + 1)]).astype(bf)
        wqk_c = np.concatenate([qc, kc], axis=1).astype(bf)
        wp_c = np.ascontiguousarray(w_proj[512 * g: 512 * (g + 1), :]).astype(bf)
        in_maps.append({
            "xT": xTb[b],
            "wqk": wqk_c,
            "wv": vc,
            "wp": wp_c,
            "cosP": cosP,
            "sinP": sinP,
            "rt": rt,
            "masks": masks,
            "ones": ones,
        })
    return in_maps


def _get_nc():
    if "nc" not in _CACHE:
        _CACHE["nc"] = _build_nc()
    return _CACHE["nc"]


def kernel(x, w_qkv, w_proj, freqs_cis, attn_mask, _trace=False):
    from concourse.bass_utils import run_bass_kernel_spmd

    in_maps = _host_prep(x, w_qkv, w_proj, freqs_cis)
    nc = _get_nc()
    res = run_bass_kernel_spmd(
        nc, in_maps, core_ids=list(range(8)), trace=_trace,
    )
    outs = [r["out"].astype(np.float64) for r in res.results]
    full = np.stack([
        outs[0] + outs[1] + outs[2] + outs[3],
        outs[4] + outs[5] + outs[6] + outs[7],
    ]).astype(np.float32)
    if _trace:
        kernel._last_results = res
    return full
